# revision 45
# baseline (speedup 1.0000x reference)
"""Causal self-attention (S=2048, B=2, D=768, H=12) on 8 TRN2 NeuronCores.

Sharding: batch*heads across cores. Core c handles batch b = c//4 and the
3 heads hs = (c%4)*3 .. hs+2. Each core computes Q/K/V projections for its
heads, causal softmax(QK^T/sqrt(hd)) @ V, and its partial contribution to
the output projection y_part = att_cat @ wc_slice^T. The host gathers by
summing the 4 per-batch partials and adding the output bias.

Key structure: the PE clock drops to 1.2GHz after any idle gap and takes
~3us of continuous work to re-reach 2.4GHz, so the emission keeps the PE
stream contiguous end to end: identity matmuls warm the clock during the
initial DMA wait; V-projection chains and head-0/1 AV chains are woven
between score chunks as fillers at a rate matched to the exp-evacuation
throughput; long av1 chains front-load phase D so the PE never stalls
while the DVE drains phase C's exp backlog. All host-side tensors are
pre-laid in partition-major SBUF layout so DMAs use large contiguous
descriptors (the head was descriptor-bound, not bandwidth-bound).
Engine routing keeps PE-gating items (kt copies, attT copies,
normalizes) at the front of the lighter queue: kt/v_sb/ys-half1
evacuations on Act, q-bias adds, ys-half0 and most AV normalizes on
DVE, causal-mask multiplies and denominator ones-columns on GpSimd (no
PSUM port). Early-needed DMAs ride the SP hwdge ring in need order
because ACT_TABLE_LOAD delays the Act ring's descriptor generation by
~3us.

Measured: 108.6-109.2us across reps (down from the 120us baseline),
PE stream >99% contiguous; phases A-C now pace at the combined Act+DVE
PSUM-evacuation (exp) throughput, which is the next wall.

Numerics: matmul inputs in bf16, accumulation in fp32 PSUM, output
partials fp32. Scores skip the max-subtraction (|s| < 3); the softmax
denominator comes from a ones-column appended to V. kt carries NO bias:
(q+bq)@bk is constant per query column, so softmax is invariant to
dropping it — kt evacuation is a plain copy on Act, in parallel with the
q bias add on DVE. Softmax exp splits between Act (native Exp) and DVE
(one-instruction Schraudolph approximation for some key blocks whose
systematic error cancels in the normalization).
"""

import numpy as np
import ml_dtypes

import concourse.bass as bass
import concourse.mybir as mybir
import concourse.tile as tile
from concourse import bacc
from concourse.bass_utils import run_bass_kernel_spmd

S = 2048  # sequence length
B = 2     # batch
D = 768   # model dim
H = 12    # heads
HD = 64   # head dim
NCORES = 8
HPC = 3   # heads per core
DC = HPC * HD          # 192: per-core head dims
VW = HPC * (HD + 1)    # 195: V columns incl per-head ones column
NQB = S // 128         # 16 query/key blocks
F32 = mybir.dt.float32
BF16 = mybir.dt.bfloat16
BF = ml_dtypes.bfloat16

TRACE = False          # set by test harness for profiled runs
LAST_RESULT = None     # BassKernelResults of the most recent run

_prog_cache = {}


def _score_chunks(kb):
    """Triangle-restricted (q0, n) chunks for key block kb, <=512 wide.

    Chunks align to the 512 grid (the first chunk of each key block may be
    short) so scores can be emitted column-region-major right behind the
    matching projection n-chunk."""
    out = []
    for j in range(kb // 4, 4):
        q0 = max(j * 512, kb * 128)
        n = (j + 1) * 512 - q0
        out.append((q0, n))
    return out


def _build_program():
    nc = bacc.Bacc()

    # all host-prepped tensors are partition-major (SBUF layout) so each
    # DMA is 128 large contiguous descriptors.
    xt = nc.declare_dram_parameter("xt", [128, 4 * 6 * 512], BF16, isOutput=False)
    wqk = nc.declare_dram_parameter("wqk", [128, 6 * 2 * DC], BF16, isOutput=False)
    bqk = nc.declare_dram_parameter("bqk", [64, 3], F32, isOutput=False)
    wv = nc.declare_dram_parameter("wv", [128, 6 * VW], BF16, isOutput=False)
    g = nc.declare_dram_parameter("g", [DC, D], BF16, isOutput=False)
    y = nc.declare_dram_parameter("y", [S, D], BF16, isOutput=True)

    with tile.TileContext(nc) as tc:
        with (
            tc.tile_pool(name="const", bufs=1) as constp,
            tc.tile_pool(name="acts", bufs=1) as actsp,
            tc.tile_pool(name="pt", bufs=2) as ptp,
            tc.tile_pool(name="small", bufs=4) as smallp,
            tc.tile_pool(name="ys", bufs=4) as ysp,
            tc.tile_pool(name="ps8", bufs=8, space="PSUM") as ps8,
        ):
            # ---- constants / weights ----
            ident = constp.tile([128, 128], BF16, tag="ident", name="ident")
            from concourse.masks import make_identity, make_upper_triangular
            make_identity(nc, ident[:])
            # mask[k, q] = 1 iff k <= q (upper triangular incl diagonal)
            mask = constp.tile([128, 128], BF16, tag="mask", name="mask")
            make_upper_triangular(nc, mask[:], val=1.0, diag=True)

            wqk_sb = constp.tile([128, 6 * 2 * DC], BF16, tag="wqk", name="wqk")
            # xt n-quarter-major: col n*3072 + k*512 + j
            xt_sb = constp.tile([128, 4 * 6 * 512], BF16, tag="xt", name="xt")

            # Everything the early pipeline needs rides the SP hwdge ring IN
            # NEED ORDER: the Act ring's descriptor generation is delayed
            # ~3us by the ACT_TABLE_LOAD hoisted to the top of the Act
            # stream, so only late-needed tensors (tiny bqk; g, needed at
            # ~85us) ride it.
            # wqk/xt-q0 in interleaved halves: the first proj chain's links
            # 1-3 start after ~0.67MB instead of the full 1.36MB; each half
            # covers 3 whole chain links so the chain never chases chunks.
            nc.sync.dma_start(wqk_sb[:, 0:1152], wqk[:, 0:1152])
            nc.sync.dma_start(xt_sb[:, 0:1536], xt[:, 0:1536])
            nc.sync.dma_start(wqk_sb[:, 1152:2304], wqk[:, 1152:2304])
            nc.sync.dma_start(xt_sb[:, 1536:3072], xt[:, 1536:3072])
            wv_sb = constp.tile([128, 6 * VW], BF16, tag="wv", name="wv")
            nc.sync.dma_start(wv_sb[:], wv[:, :])
            # bias columns: col h = bq_h*sc (kt is bias-free, see docstring)
            bqk_sb = constp.tile([64, 3], F32, tag="bqk", name="bqk")
            nc.scalar.dma_start(bqk_sb[:], bqk[:, :])
            nc.sync.dma_start(xt_sb[:, 3072:6144], xt[:, 3072:6144])
            nc.sync.dma_start(xt_sb[:, 6144:9216], xt[:, 6144:9216])
            nc.sync.dma_start(xt_sb[:, 9216:12288], xt[:, 9216:12288])
            # g rows 0:128 at base partition 0; rows 128:192 parked at base
            # partition 64 so yproj's second matmul (lhsT = attT1[64:128])
            # sees both operands at the same base partition.
            g_sb0 = constp.tile([128, D], BF16, tag="g0", name="g0")
            nc.scalar.dma_start(g_sb0[:], g[0:128, :])
            g_sb1 = constp.tile([128, D], BF16, tag="g1", name="g1")
            nc.scalar.dma_start(g_sb1[64:128, :], g[128:192, :])

            def xt_nk(n, k):
                base = n * 3072 + k * 512
                return xt_sb[:, base:base + 512]

            def wqkc(k):
                return wqk_sb[:, k * 2 * DC:(k + 1) * 2 * DC]

            def wvc(k):
                return wv_sb[:, k * VW:(k + 1) * VW]

            # ---- activations ----
            # wqk column band h*128..h*128+128 holds [q_h | k_h] (64 each)
            qt = [actsp.tile([64, S], BF16, tag=f"qt{h}", name=f"qt{h}")
                  for h in range(HPC)]
            kt = [actsp.tile([64, S], BF16, tag=f"kt{h}", name=f"kt{h}")
                  for h in range(HPC)]
            v_sb = [actsp.tile([128, VW], BF16, tag=f"v{kb}", name=f"v{kb}")
                    for kb in range(NQB)]
            att3 = [actsp.tile([128, DC], BF16, tag=f"att{qi}", name=f"att{qi}")
                    for qi in range(NQB)]
            attT0 = actsp.tile([128, S], BF16, tag="attT0", name="attT0")
            # head-2 rows parked at partitions 64:128 to match g_sb1
            attT1 = actsp.tile([128, S], BF16, tag="attT1", name="attT1")

            # ---- emission helpers ----
            def emit_proj(h, ns=range(4)):
                """Q/K projection for head h -> qt[h] (+bias, DVE), kt[h]
                (no bias, plain copy on Act; kt first: it gates the score
                LDWEIGHTS)."""
                for n in ns:
                    ps = ps8.tile([128, 512], F32, tag="ps", name="psqk")
                    for k in range(6):
                        nc.tensor.matmul(
                            ps[:], wqkc(k)[:, h * 128:(h + 1) * 128],
                            xt_nk(n, k),
                            start=(k == 0), stop=(k == 5))
                    nc.scalar.copy(
                        kt[h][:, n * 512:(n + 1) * 512], ps[64:128, :])
                    nc.vector.tensor_scalar_add(
                        qt[h][:, n * 512:(n + 1) * 512], ps[0:64, :],
                        bqk_sb[:, h:h + 1])

            def emit_vproj(kb):
                """V projection for key block kb -> v_sb[kb].

                No bias: +bv flows through the output projection as the
                constant bv @ wc.T, which the host folds into bc. The
                denominator ones-columns are memset separately."""
                n, o = kb // 4, (kb % 4) * 128
                ps = ps8.tile([128, 512], F32, tag="ps", name="psv")
                for k in range(6):
                    nc.tensor.matmul(
                        ps[:, 0:VW], xt_nk(n, k)[:, o:o + 128],
                        wvc(k)[:], start=(k == 0), stop=(k == 5))
                # Act evac: keeps the DVE free for the q bias adds that gate
                # the score stream (Act order: kt copies first, then this).
                nc.scalar.copy(v_sb[kb][:], ps[:, 0:VW])
                ones_ap = v_sb[kb][:].rearrange("p (h j) -> p h j", h=3)[:, :, 64:65]
                nc.gpsimd.memset(ones_ap, 1.0)

            pt = {}  # (h, kb) -> pt tile

            # Key blocks whose exp runs on DVE as a one-instruction
            # Schraudolph approximation: bf16(exp(s)) ~= bitcast(int16(
            # s*128/ln2 + B)); the systematic error largely cancels in the
            # softmax normalization.
            APPROX_KB = (2, 4, 5, 6, 8, 10, 12, 13)
            EXP_A = 184.6650   # 2^7 / ln 2
            EXP_B = 16247.75   # 127*2^7 minus Schraudolph bias correction

            def emit_score_chunk(h, kb, q0, n):
                t = pt[(h, kb)]
                ps = ps8.tile([128, 512], F32, tag="ps", name="psmm")
                nc.tensor.matmul(
                    ps[:, :n], kt[h][:, kb * 128:(kb + 1) * 128],
                    qt[h][:, q0:q0 + n], start=True, stop=True)
                dst = t[:, q0 - kb * 128:q0 - kb * 128 + n]
                if kb in APPROX_KB:
                    nc.vector.tensor_scalar(
                        dst.bitcast(mybir.dt.int16), ps[:, :n],
                        EXP_A, EXP_B,
                        mybir.AluOpType.mult, mybir.AluOpType.add)
                else:
                    nc.scalar.activation(
                        dst, ps[:, :n], mybir.ActivationFunctionType.Exp)

            def emit_mask(h, kb):
                t = pt[(h, kb)]
                nc.gpsimd.tensor_mul(t[:, 0:128], t[:, 0:128], mask[:])

            def alloc_pt(h, kb):
                t = ptp.tile([128, S - kb * 128], BF16, tag=f"pt{kb}",
                             name=f"pt{kb}", bufs=3)
                pt[(h, kb)] = t

            def emit_av(h, qi):
                """AV for (head h, query block qi) -> normalized att3 cols."""
                po = ps8.tile([128, HD + 1], F32, tag="ps", name="po")
                for kb in range(qi + 1):
                    nc.tensor.matmul(
                        po[:], pt[(h, kb)][:, (qi - kb) * 128:(qi - kb + 1) * 128],
                        v_sb[kb][:, h * 65:h * 65 + 65],
                        start=(kb == 0), stop=(kb == qi))
                r = smallp.tile([128, 1], F32, tag="r", name="r")
                nc.vector.reciprocal_approx_fast(r[:], po[:, HD:HD + 1])
                dst = att3[qi][:, h * 64:(h + 1) * 64]
                if (h, qi) in SCAL_NORM:
                    nc.scalar.mul(dst, po[:, 0:HD], r[:])
                else:
                    nc.vector.tensor_scalar_mul(dst, po[:, 0:HD], r[:])

            # normalizes routed to Act where the DVE is the busier engine
            SCAL_NORM = {(2, qi) for qi in range(8, NQB)}

            def emit_transp(qi):
                """Transpose att3[qi] on the PE, evacuate to attT0/attT1.

                (DMA xbar transposes are semantically correct here but cost
                ~1.3us each on real hardware vs the model's 14ns/tile —
                measured +42us end-to-end — so they stay on the PE.)"""
                tr = ps8.tile([128, 256], BF16, tag="ps", name="tr")
                nc.tensor.transpose(tr[:, 0:128], att3[qi][:, 0:128], ident[:])
                nc.tensor.transpose(tr[0:64, 128:256], att3[qi][:, 128:192],
                                    ident[:])
                nc.vector.tensor_copy(attT0[:, qi * 128:(qi + 1) * 128],
                                      tr[:, 0:128])
                if qi >= 6:
                    nc.scalar.copy(attT1[64:128, qi * 128:(qi + 1) * 128],
                                   tr[0:64, 128:256])
                else:
                    nc.vector.tensor_copy(
                        attT1[64:128, qi * 128:(qi + 1) * 128],
                        tr[0:64, 128:256])

            def emit_yproj(qi):
                """Output projection for query block qi and DMA out."""
                ys = ysp.tile([128, D], BF16, tag="y", name="ys")
                tailp = qi >= 13
                for half in range(2):
                    ps = ps8.tile([128, 384], F32, tag="ps", name="psyp")
                    nc.tensor.matmul(
                        ps[:], attT0[:, qi * 128:(qi + 1) * 128],
                        g_sb0[:, half * 384:(half + 1) * 384],
                        start=True, stop=False)
                    nc.tensor.matmul(
                        ps[:], attT1[64:128, qi * 128:(qi + 1) * 128],
                        g_sb1[64:128, half * 384:(half + 1) * 384],
                        start=False, stop=True)
                    dst = ys[:, half * 384:(half + 1) * 384]
                    # one half per engine: with two steps of transpose slack
                    # the ys copies no longer delay the next step's attT
                    # copies, so split them evenly
                    if half == 1:
                        nc.scalar.copy(dst, ps[:])
                    else:
                        nc.vector.tensor_copy(dst, ps[:])
                    if tailp:
                        (nc.sync if half == 0 else nc.scalar).dma_start(
                            y[qi * 128:(qi + 1) * 128,
                              half * 384:(half + 1) * 384],
                            ys[:, half * 384:(half + 1) * 384])
                if not tailp:
                    nc.sync.dma_start(y[qi * 128:(qi + 1) * 128, :], ys[:])

            # ---- emission: contiguous PE stream with fillers ----
            for kb in range(NQB):
                alloc_pt(0, kb)
                alloc_pt(1, kb)

            vq = list(range(NQB))       # pending V projections (fillers)
            av0q = list(range(NQB))     # pending head-0 AV chains (fillers)

            def vfill(k=1):
                for _ in range(k):
                    if vq:
                        emit_vproj(vq.pop(0))

            def av0fill(limit, k=1):
                for _ in range(k):
                    if av0q and av0q[0] < limit:
                        emit_av(0, av0q.pop(0))

            av1q = list(range(12))  # consumed longest-chain first via pop()

            def fill(av0_limit, av1_limit=-1):
                """One filler unit: an av0/av1 chain whose inputs are at
                least a region old, else a pending V projection chain."""
                if av0q and av0q[0] < av0_limit:
                    emit_av(0, av0q.pop(0))
                elif vq:
                    vfill(1)
                elif av1q and av1q[-1] < av1_limit:
                    emit_av(1, av1q.pop())

            # PE clock warmup: the PE ramps 0.65 -> 1.2 -> 2.4GHz over ~3us
            # of continuous execution, so burn identity matmuls during the
            # initial DMA wait; the first real chains then run at full rate.
            wps = ps8.tile([128, 512], F32, tag="ps", name="warm")
            for _ in range(52):
                nc.tensor.matmul(wps[:, 0:128], ident[:], ident[:],
                                 start=True, stop=True)
            scrap = smallp.tile([128, 128], F32, tag="scrap", name="scrap")
            nc.vector.tensor_copy(scrap[:], wps[:, 0:128])

            # Phases A/B: per column-region n, both heads' proj chains, then
            # head-0 scores, then head-1 scores, fillers every 3 chunks.
            for n in range(4):
                emit_proj(0, ns=(n,))
                emit_proj(1, ns=(n,))
                vfill(2 if n == 0 else 1)
                if n >= 2:
                    fill(4 * n, 4 * n)
                for h in (0, 1):
                    cnt = 0
                    lim = 4 * n if h == 0 else 4 * n + 4
                    for kb in range(4 * n + 4):
                        for (q0, c) in _score_chunks(kb):
                            if q0 // 512 == n:
                                emit_score_chunk(h, kb, q0, c)
                                cnt += 1
                                if cnt % 3 == 0:
                                    fill(lim, 4 * n)
                        if kb // 4 == n:
                            emit_mask(h, kb)
                    fill(lim, 4 * n)

            # Phase C: head-2 proj + scores; leftover av0/V plus av1 chains
            # (longest first) as fillers.
            def cfill(k=1):
                for _ in range(k):
                    fill(NQB, NQB)
            emit_proj(2, ns=(0,))
            emit_proj(2, ns=(1,))
            cfill(2)
            emit_proj(2, ns=(2,))
            cfill(2)
            emit_proj(2, ns=(3,))
            cfill(2)
            cnt = 0
            for n in range(4):
                for kb in range(4 * n + 4):
                    if (2, kb) not in pt:
                        alloc_pt(2, kb)
                    for (q0, c) in _score_chunks(kb):
                        if q0 // 512 == n:
                            emit_score_chunk(2, kb, q0, c)
                            cnt += 1
                            if cnt % 3 == 0:
                                cfill(1)
                    if kb // 4 == n:
                        emit_mask(2, kb)
            while av0q or vq or av1q:
                cfill(1)

            # Phase D: remaining AV; per-qi tail pipeline with one step of
            # slack between normalize -> transp -> yproj. The av1(12..15)
            # chains front-load D so the PE stays busy while the DVE drains
            # phase C's exp backlog (a stall here resets the PE clock and
            # poisons the whole tail).
            # av2(kb) at step kb (its inputs are all ready at D start) gives
            # the transposes two steps of normalize slack instead of one.
            for kb in range(NQB):
                if kb < 4:
                    emit_av(1, 12 + kb)
                emit_av(2, kb)
                if kb >= 2:
                    emit_transp(kb - 2)
                if kb >= 3:
                    emit_yproj(kb - 3)
            emit_transp(NQB - 2)
            emit_yproj(NQB - 3)
            emit_transp(NQB - 1)
            emit_yproj(NQB - 2)
            emit_yproj(NQB - 1)

    nc.finalize()
    return nc


def _prep_inputs(x, wq, bq, wk, bk, wv, bv, wc, bc):
    """Per-core input maps, all host-side slicing/transposition.

    Everything is pre-laid in partition-major SBUF layout so device DMAs
    use large contiguous descriptors."""
    sc = 1.0 / np.sqrt(np.float32(HD))
    in_maps = []
    for c in range(NCORES):
        b = c // 4
        r0 = (c % 4) * HPC * HD
        xtr = np.ascontiguousarray(x[:, b, :].T)  # [768, 2048]
        # -> [128, (n k j)]: col n*3072 + k*512 + j <- xtr[k*128+p, n*512+j]
        xt_h = xtr.reshape(6, 128, 4, 512).transpose(1, 2, 0, 3).reshape(128, 12288)
        # wqk columns: per-head bands [q_h | k_h] (64 each)
        wqk_cols = []
        bqk_cols = []
        for j in range(HPC):
            hr = slice(r0 + j * HD, r0 + (j + 1) * HD)
            wqk_cols.append(wq[hr] * sc)
            wqk_cols.append(wk[hr])
            bqk_cols.append(bq[hr] * sc)
        wqk_f = np.concatenate(wqk_cols, axis=0).T  # [768, 384]
        wqk_h = wqk_f.reshape(6, 128, 2 * DC).transpose(1, 0, 2).reshape(128, 6 * 2 * DC)
        bqk_t = np.stack(bqk_cols, axis=1).astype(np.float32)  # [64, 3]
        wva = np.zeros((D, VW), np.float32)
        for j in range(HPC):
            hr = slice(r0 + j * HD, r0 + (j + 1) * HD)
            wva[:D, j * 65:j * 65 + HD] = wv[hr].T
        wv_h = wva.reshape(6, 128, VW).transpose(1, 0, 2).reshape(128, 6 * VW)
        rows = slice(r0, r0 + DC)
        g = np.ascontiguousarray(wc[:, rows].T).astype(BF)
        in_maps.append({
            "xt": np.ascontiguousarray(xt_h).astype(BF),
            "wqk": np.ascontiguousarray(wqk_h).astype(BF),
            "bqk": bqk_t,
            "wv": np.ascontiguousarray(wv_h).astype(BF),
            "g": g,
        })
    return in_maps


def kernel(**inputs):
    global LAST_RESULT
    if "prog" not in _prog_cache:
        _prog_cache["prog"] = _build_program()
    nc = _prog_cache["prog"]

    args = {k: np.asarray(inputs[k], np.float32)
            for k in ("x", "wq", "bq", "wk", "bk", "wv", "bv", "wc", "bc")}
    in_maps = _prep_inputs(**args)
    res = run_bass_kernel_spmd(nc, in_maps, core_ids=list(range(NCORES)),
                               trace=TRACE)
    LAST_RESULT = res

    # V-bias contribution: att gets +bv per head dim, so y gets +bv @ wc.T
    bc_eff = args["bc"] + args["bv"] @ args["wc"].T
    out = np.empty((S, B, D), np.float32)
    for b in range(B):
        acc = res.results[4 * b]["y"].astype(np.float32)
        for c in range(4 * b + 1, 4 * b + 4):
            acc = acc + res.results[c]["y"]
        out[:, b, :] = acc + bc_eff[None, :]
    return out


# revision 46
# speedup vs baseline: 1.0011x; 1.0011x over previous
"""Causal self-attention (S=2048, B=2, D=768, H=12) on 8 TRN2 NeuronCores.

Sharding: batch*heads across cores. Core c handles batch b = c//4 and the
3 heads hs = (c%4)*3 .. hs+2. Each core computes Q/K/V projections for its
heads, causal softmax(QK^T/sqrt(hd)) @ V, and its partial contribution to
the output projection y_part = att_cat @ wc_slice^T. The host gathers by
summing the 4 per-batch partials and adding the output bias.

Key structure: the PE clock drops to 1.2GHz after any idle gap and takes
~3us of continuous work to re-reach 2.4GHz, so the emission keeps the PE
stream contiguous end to end: identity matmuls warm the clock during the
initial DMA wait; V-projection chains and head-0/1 AV chains are woven
between score chunks as fillers at a rate matched to the exp-evacuation
throughput; long av1 chains front-load phase D so the PE never stalls
while the DVE drains phase C's exp backlog. All host-side tensors are
pre-laid in partition-major SBUF layout so DMAs use large contiguous
descriptors (the head was descriptor-bound, not bandwidth-bound).
Engine routing keeps PE-gating items (kt copies, attT copies,
normalizes) at the front of the lighter queue: kt/v_sb/ys-half1
evacuations on Act, q-bias adds, ys-half0 and most AV normalizes on
DVE, causal-mask multiplies and denominator ones-columns on GpSimd (no
PSUM port). Early-needed DMAs ride the SP hwdge ring in need order
because ACT_TABLE_LOAD delays the Act ring's descriptor generation by
~3us.

Measured: 108.6-109.2us across reps (down from the 120us baseline),
PE stream >99% contiguous; phases A-C now pace at the combined Act+DVE
PSUM-evacuation (exp) throughput, which is the next wall.

Numerics: matmul inputs in bf16, accumulation in fp32 PSUM, output
partials fp32. Scores skip the max-subtraction (|s| < 3); the softmax
denominator comes from a ones-column appended to V. kt carries NO bias:
(q+bq)@bk is constant per query column, so softmax is invariant to
dropping it — kt evacuation is a plain copy on Act, in parallel with the
q bias add on DVE. Softmax exp splits between Act (native Exp) and DVE
(one-instruction Schraudolph approximation for some key blocks whose
systematic error cancels in the normalization).
"""

import numpy as np
import ml_dtypes

import concourse.bass as bass
import concourse.mybir as mybir
import concourse.tile as tile
from concourse import bacc
from concourse.bass_utils import run_bass_kernel_spmd

S = 2048  # sequence length
B = 2     # batch
D = 768   # model dim
H = 12    # heads
HD = 64   # head dim
NCORES = 8
HPC = 3   # heads per core
DC = HPC * HD          # 192: per-core head dims
VW = HPC * (HD + 1)    # 195: V columns incl per-head ones column
NQB = S // 128         # 16 query/key blocks
F32 = mybir.dt.float32
BF16 = mybir.dt.bfloat16
BF = ml_dtypes.bfloat16

TRACE = False          # set by test harness for profiled runs
LAST_RESULT = None     # BassKernelResults of the most recent run

_prog_cache = {}


def _score_chunks(kb):
    """Triangle-restricted (q0, n) chunks for key block kb, <=512 wide.

    Chunks align to the 512 grid (the first chunk of each key block may be
    short) so scores can be emitted column-region-major right behind the
    matching projection n-chunk."""
    out = []
    for j in range(kb // 4, 4):
        q0 = max(j * 512, kb * 128)
        n = (j + 1) * 512 - q0
        out.append((q0, n))
    return out


def _build_program():
    nc = bacc.Bacc()

    # all host-prepped tensors are partition-major (SBUF layout) so each
    # DMA is 128 large contiguous descriptors.
    xt = nc.declare_dram_parameter("xt", [128, 4 * 6 * 512], BF16, isOutput=False)
    wqk = nc.declare_dram_parameter("wqk", [128, 6 * 2 * DC], BF16, isOutput=False)
    bqk = nc.declare_dram_parameter("bqk", [64, 3], F32, isOutput=False)
    wv = nc.declare_dram_parameter("wv", [128, 6 * VW], BF16, isOutput=False)
    g = nc.declare_dram_parameter("g", [DC, D], BF16, isOutput=False)
    y = nc.declare_dram_parameter("y", [S, D], BF16, isOutput=True)

    with tile.TileContext(nc) as tc:
        with (
            tc.tile_pool(name="const", bufs=1) as constp,
            tc.tile_pool(name="acts", bufs=1) as actsp,
            tc.tile_pool(name="pt", bufs=2) as ptp,
            tc.tile_pool(name="small", bufs=4) as smallp,
            tc.tile_pool(name="ys", bufs=4) as ysp,
            tc.tile_pool(name="ps8", bufs=8, space="PSUM") as ps8,
        ):
            # ---- constants / weights ----
            ident = constp.tile([128, 128], BF16, tag="ident", name="ident")
            from concourse.masks import make_identity, make_upper_triangular
            make_identity(nc, ident[:])
            # mask[k, q] = 1 iff k <= q (upper triangular incl diagonal)
            mask = constp.tile([128, 128], BF16, tag="mask", name="mask")
            make_upper_triangular(nc, mask[:], val=1.0, diag=True)

            wqk_sb = constp.tile([128, 6 * 2 * DC], BF16, tag="wqk", name="wqk")
            # xt n-quarter-major: col n*3072 + k*512 + j
            xt_sb = constp.tile([128, 4 * 6 * 512], BF16, tag="xt", name="xt")

            # Everything the early pipeline needs rides the SP hwdge ring IN
            # NEED ORDER: the Act ring's descriptor generation is delayed
            # ~3us by the ACT_TABLE_LOAD hoisted to the top of the Act
            # stream, so only late-needed tensors (tiny bqk; g, needed at
            # ~85us) ride it.
            # wqk/xt-q0 in interleaved halves: the first proj chain's links
            # 1-3 start after ~0.67MB instead of the full 1.36MB; each half
            # covers 3 whole chain links so the chain never chases chunks.
            nc.sync.dma_start(wqk_sb[:, 0:1152], wqk[:, 0:1152])
            nc.sync.dma_start(xt_sb[:, 0:1536], xt[:, 0:1536])
            nc.sync.dma_start(wqk_sb[:, 1152:2304], wqk[:, 1152:2304])
            nc.sync.dma_start(xt_sb[:, 1536:3072], xt[:, 1536:3072])
            wv_sb = constp.tile([128, 6 * VW], BF16, tag="wv", name="wv")
            nc.sync.dma_start(wv_sb[:], wv[:, :])
            # bias columns: col h = bq_h*sc (kt is bias-free, see docstring)
            bqk_sb = constp.tile([64, 3], F32, tag="bqk", name="bqk")
            nc.scalar.dma_start(bqk_sb[:], bqk[:, :])
            nc.sync.dma_start(xt_sb[:, 3072:6144], xt[:, 3072:6144])
            nc.sync.dma_start(xt_sb[:, 6144:9216], xt[:, 6144:9216])
            nc.sync.dma_start(xt_sb[:, 9216:12288], xt[:, 9216:12288])
            # g rows 0:128 at base partition 0; rows 128:192 parked at base
            # partition 64 so yproj's second matmul (lhsT = attT1[64:128])
            # sees both operands at the same base partition.
            g_sb0 = constp.tile([128, D], BF16, tag="g0", name="g0")
            nc.scalar.dma_start(g_sb0[:], g[0:128, :])
            g_sb1 = constp.tile([128, D], BF16, tag="g1", name="g1")
            nc.scalar.dma_start(g_sb1[64:128, :], g[128:192, :])

            def xt_nk(n, k):
                base = n * 3072 + k * 512
                return xt_sb[:, base:base + 512]

            def wqkc(k):
                return wqk_sb[:, k * 2 * DC:(k + 1) * 2 * DC]

            def wvc(k):
                return wv_sb[:, k * VW:(k + 1) * VW]

            # ---- activations ----
            # wqk column band h*128..h*128+128 holds [q_h | k_h] (64 each)
            qt = [actsp.tile([64, S], BF16, tag=f"qt{h}", name=f"qt{h}")
                  for h in range(HPC)]
            kt = [actsp.tile([64, S], BF16, tag=f"kt{h}", name=f"kt{h}")
                  for h in range(HPC)]
            v_sb = [actsp.tile([128, VW], BF16, tag=f"v{kb}", name=f"v{kb}")
                    for kb in range(NQB)]
            att3 = [actsp.tile([128, DC], BF16, tag=f"att{qi}", name=f"att{qi}")
                    for qi in range(NQB)]
            attT0 = actsp.tile([128, S], BF16, tag="attT0", name="attT0")
            # head-2 rows parked at partitions 64:128 to match g_sb1
            attT1 = actsp.tile([128, S], BF16, tag="attT1", name="attT1")

            # ---- emission helpers ----
            def emit_proj(h, ns=range(4)):
                """Q/K projection for head h -> qt[h] (+bias, DVE), kt[h]
                (no bias, plain copy on Act; kt first: it gates the score
                LDWEIGHTS)."""
                for n in ns:
                    ps = ps8.tile([128, 512], F32, tag="ps", name="psqk")
                    for k in range(6):
                        nc.tensor.matmul(
                            ps[:], wqkc(k)[:, h * 128:(h + 1) * 128],
                            xt_nk(n, k),
                            start=(k == 0), stop=(k == 5))
                    nc.scalar.copy(
                        kt[h][:, n * 512:(n + 1) * 512], ps[64:128, :])
                    nc.vector.tensor_scalar_add(
                        qt[h][:, n * 512:(n + 1) * 512], ps[0:64, :],
                        bqk_sb[:, h:h + 1])

            def emit_vproj(kb):
                """V projection for key block kb -> v_sb[kb].

                No bias: +bv flows through the output projection as the
                constant bv @ wc.T, which the host folds into bc. The
                denominator ones-columns are memset separately."""
                n, o = kb // 4, (kb % 4) * 128
                ps = ps8.tile([128, 512], F32, tag="ps", name="psv")
                for k in range(6):
                    nc.tensor.matmul(
                        ps[:, 0:VW], xt_nk(n, k)[:, o:o + 128],
                        wvc(k)[:], start=(k == 0), stop=(k == 5))
                # Act evac: keeps the DVE free for the q bias adds that gate
                # the score stream (Act order: kt copies first, then this).
                nc.scalar.copy(v_sb[kb][:], ps[:, 0:VW])
                ones_ap = v_sb[kb][:].rearrange("p (h j) -> p h j", h=3)[:, :, 64:65]
                nc.gpsimd.memset(ones_ap, 1.0)

            pt = {}  # (h, kb) -> pt tile

            # Key blocks whose exp runs on DVE as a one-instruction
            # Schraudolph approximation: bf16(exp(s)) ~= bitcast(int16(
            # s*128/ln2 + B)); the systematic error largely cancels in the
            # softmax normalization.
            APPROX_KB = (2, 4, 5, 6, 8, 10, 12)
            EXP_A = 184.6650   # 2^7 / ln 2
            EXP_B = 16247.75   # 127*2^7 minus Schraudolph bias correction

            def emit_score_chunk(h, kb, q0, n):
                t = pt[(h, kb)]
                ps = ps8.tile([128, 512], F32, tag="ps", name="psmm")
                nc.tensor.matmul(
                    ps[:, :n], kt[h][:, kb * 128:(kb + 1) * 128],
                    qt[h][:, q0:q0 + n], start=True, stop=True)
                dst = t[:, q0 - kb * 128:q0 - kb * 128 + n]
                if kb in APPROX_KB:
                    nc.vector.tensor_scalar(
                        dst.bitcast(mybir.dt.int16), ps[:, :n],
                        EXP_A, EXP_B,
                        mybir.AluOpType.mult, mybir.AluOpType.add)
                else:
                    nc.scalar.activation(
                        dst, ps[:, :n], mybir.ActivationFunctionType.Exp)

            def emit_mask(h, kb):
                t = pt[(h, kb)]
                nc.gpsimd.tensor_mul(t[:, 0:128], t[:, 0:128], mask[:])

            def alloc_pt(h, kb):
                t = ptp.tile([128, S - kb * 128], BF16, tag=f"pt{kb}",
                             name=f"pt{kb}", bufs=3)
                pt[(h, kb)] = t

            def emit_av(h, qi):
                """AV for (head h, query block qi) -> normalized att3 cols."""
                po = ps8.tile([128, HD + 1], F32, tag="ps", name="po")
                for kb in range(qi + 1):
                    nc.tensor.matmul(
                        po[:], pt[(h, kb)][:, (qi - kb) * 128:(qi - kb + 1) * 128],
                        v_sb[kb][:, h * 65:h * 65 + 65],
                        start=(kb == 0), stop=(kb == qi))
                r = smallp.tile([128, 1], F32, tag="r", name="r")
                nc.vector.reciprocal_approx_fast(r[:], po[:, HD:HD + 1])
                dst = att3[qi][:, h * 64:(h + 1) * 64]
                if (h, qi) in SCAL_NORM:
                    nc.scalar.mul(dst, po[:, 0:HD], r[:])
                else:
                    nc.vector.tensor_scalar_mul(dst, po[:, 0:HD], r[:])

            # normalizes routed to Act where the DVE is the busier engine
            SCAL_NORM = {(2, qi) for qi in range(8, NQB)}

            def emit_transp(qi):
                """Transpose att3[qi] on the PE, evacuate to attT0/attT1.

                (DMA xbar transposes are semantically correct here but cost
                ~1.3us each on real hardware vs the model's 14ns/tile —
                measured +42us end-to-end — so they stay on the PE.)"""
                tr = ps8.tile([128, 256], BF16, tag="ps", name="tr")
                nc.tensor.transpose(tr[:, 0:128], att3[qi][:, 0:128], ident[:])
                nc.tensor.transpose(tr[0:64, 128:256], att3[qi][:, 128:192],
                                    ident[:])
                nc.vector.tensor_copy(attT0[:, qi * 128:(qi + 1) * 128],
                                      tr[:, 0:128])
                if qi >= 6:
                    nc.scalar.copy(attT1[64:128, qi * 128:(qi + 1) * 128],
                                   tr[0:64, 128:256])
                else:
                    nc.vector.tensor_copy(
                        attT1[64:128, qi * 128:(qi + 1) * 128],
                        tr[0:64, 128:256])

            def emit_yproj(qi):
                """Output projection for query block qi and DMA out."""
                ys = ysp.tile([128, D], BF16, tag="y", name="ys")
                tailp = qi >= 13
                for half in range(2):
                    ps = ps8.tile([128, 384], F32, tag="ps", name="psyp")
                    nc.tensor.matmul(
                        ps[:], attT0[:, qi * 128:(qi + 1) * 128],
                        g_sb0[:, half * 384:(half + 1) * 384],
                        start=True, stop=False)
                    nc.tensor.matmul(
                        ps[:], attT1[64:128, qi * 128:(qi + 1) * 128],
                        g_sb1[64:128, half * 384:(half + 1) * 384],
                        start=False, stop=True)
                    dst = ys[:, half * 384:(half + 1) * 384]
                    # one half per engine: with two steps of transpose slack
                    # the ys copies no longer delay the next step's attT
                    # copies, so split them evenly
                    if half == 1:
                        nc.scalar.copy(dst, ps[:])
                    else:
                        nc.vector.tensor_copy(dst, ps[:])
                    if tailp:
                        (nc.sync if half == 0 else nc.scalar).dma_start(
                            y[qi * 128:(qi + 1) * 128,
                              half * 384:(half + 1) * 384],
                            ys[:, half * 384:(half + 1) * 384])
                if not tailp:
                    nc.sync.dma_start(y[qi * 128:(qi + 1) * 128, :], ys[:])

            # ---- emission: contiguous PE stream with fillers ----
            for kb in range(NQB):
                alloc_pt(0, kb)
                alloc_pt(1, kb)

            vq = list(range(NQB))       # pending V projections (fillers)
            av0q = list(range(NQB))     # pending head-0 AV chains (fillers)

            def vfill(k=1):
                for _ in range(k):
                    if vq:
                        emit_vproj(vq.pop(0))

            def av0fill(limit, k=1):
                for _ in range(k):
                    if av0q and av0q[0] < limit:
                        emit_av(0, av0q.pop(0))

            av1q = list(range(12))  # consumed longest-chain first via pop()

            def fill(av0_limit, av1_limit=-1):
                """One filler unit: an av0/av1 chain whose inputs are at
                least a region old, else a pending V projection chain."""
                if av0q and av0q[0] < av0_limit:
                    emit_av(0, av0q.pop(0))
                elif vq:
                    vfill(1)
                elif av1q and av1q[-1] < av1_limit:
                    emit_av(1, av1q.pop())

            # PE clock warmup: the PE ramps 0.65 -> 1.2 -> 2.4GHz over ~3us
            # of continuous execution, so burn identity matmuls during the
            # initial DMA wait; the first real chains then run at full rate.
            wps = ps8.tile([128, 512], F32, tag="ps", name="warm")
            for _ in range(52):
                nc.tensor.matmul(wps[:, 0:128], ident[:], ident[:],
                                 start=True, stop=True)
            scrap = smallp.tile([128, 128], F32, tag="scrap", name="scrap")
            nc.vector.tensor_copy(scrap[:], wps[:, 0:128])

            # Phases A/B: per column-region n, both heads' proj chains, then
            # head-0 scores, then head-1 scores, fillers every 3 chunks.
            for n in range(4):
                emit_proj(0, ns=(n,))
                emit_proj(1, ns=(n,))
                vfill(2 if n == 0 else 1)
                if n >= 2:
                    fill(4 * n, 4 * n)
                for h in (0, 1):
                    cnt = 0
                    lim = 4 * n if h == 0 else 4 * n + 4
                    for kb in range(4 * n + 4):
                        for (q0, c) in _score_chunks(kb):
                            if q0 // 512 == n:
                                emit_score_chunk(h, kb, q0, c)
                                cnt += 1
                                if cnt % 3 == 0:
                                    fill(lim, 4 * n)
                        if kb // 4 == n:
                            emit_mask(h, kb)
                    fill(lim, 4 * n)

            # Phase C: head-2 proj + scores; leftover av0/V plus av1 chains
            # (longest first) as fillers.
            def cfill(k=1):
                for _ in range(k):
                    fill(NQB, NQB)
            emit_proj(2, ns=(0,))
            emit_proj(2, ns=(1,))
            cfill(2)
            emit_proj(2, ns=(2,))
            cfill(2)
            emit_proj(2, ns=(3,))
            cfill(2)
            cnt = 0
            for n in range(4):
                for kb in range(4 * n + 4):
                    if (2, kb) not in pt:
                        alloc_pt(2, kb)
                    for (q0, c) in _score_chunks(kb):
                        if q0 // 512 == n:
                            emit_score_chunk(2, kb, q0, c)
                            cnt += 1
                            if cnt % 3 == 0:
                                cfill(1)
                    if kb // 4 == n:
                        emit_mask(2, kb)
            while av0q or vq or av1q:
                cfill(1)

            # Phase D: remaining AV; per-qi tail pipeline with one step of
            # slack between normalize -> transp -> yproj. The av1(12..15)
            # chains front-load D so the PE stays busy while the DVE drains
            # phase C's exp backlog (a stall here resets the PE clock and
            # poisons the whole tail).
            # av2(kb) at step kb (its inputs are all ready at D start) gives
            # the transposes two steps of normalize slack instead of one.
            for kb in range(NQB):
                if kb < 4:
                    emit_av(1, 12 + kb)
                emit_av(2, kb)
                if kb >= 2:
                    emit_transp(kb - 2)
                if kb >= 3:
                    emit_yproj(kb - 3)
            emit_transp(NQB - 2)
            emit_yproj(NQB - 3)
            emit_transp(NQB - 1)
            emit_yproj(NQB - 2)
            emit_yproj(NQB - 1)

    nc.finalize()
    return nc


def _prep_inputs(x, wq, bq, wk, bk, wv, bv, wc, bc):
    """Per-core input maps, all host-side slicing/transposition.

    Everything is pre-laid in partition-major SBUF layout so device DMAs
    use large contiguous descriptors."""
    sc = 1.0 / np.sqrt(np.float32(HD))
    in_maps = []
    for c in range(NCORES):
        b = c // 4
        r0 = (c % 4) * HPC * HD
        xtr = np.ascontiguousarray(x[:, b, :].T)  # [768, 2048]
        # -> [128, (n k j)]: col n*3072 + k*512 + j <- xtr[k*128+p, n*512+j]
        xt_h = xtr.reshape(6, 128, 4, 512).transpose(1, 2, 0, 3).reshape(128, 12288)
        # wqk columns: per-head bands [q_h | k_h] (64 each)
        wqk_cols = []
        bqk_cols = []
        for j in range(HPC):
            hr = slice(r0 + j * HD, r0 + (j + 1) * HD)
            wqk_cols.append(wq[hr] * sc)
            wqk_cols.append(wk[hr])
            bqk_cols.append(bq[hr] * sc)
        wqk_f = np.concatenate(wqk_cols, axis=0).T  # [768, 384]
        wqk_h = wqk_f.reshape(6, 128, 2 * DC).transpose(1, 0, 2).reshape(128, 6 * 2 * DC)
        bqk_t = np.stack(bqk_cols, axis=1).astype(np.float32)  # [64, 3]
        wva = np.zeros((D, VW), np.float32)
        for j in range(HPC):
            hr = slice(r0 + j * HD, r0 + (j + 1) * HD)
            wva[:D, j * 65:j * 65 + HD] = wv[hr].T
        wv_h = wva.reshape(6, 128, VW).transpose(1, 0, 2).reshape(128, 6 * VW)
        rows = slice(r0, r0 + DC)
        g = np.ascontiguousarray(wc[:, rows].T).astype(BF)
        in_maps.append({
            "xt": np.ascontiguousarray(xt_h).astype(BF),
            "wqk": np.ascontiguousarray(wqk_h).astype(BF),
            "bqk": bqk_t,
            "wv": np.ascontiguousarray(wv_h).astype(BF),
            "g": g,
        })
    return in_maps


def kernel(**inputs):
    global LAST_RESULT
    if "prog" not in _prog_cache:
        _prog_cache["prog"] = _build_program()
    nc = _prog_cache["prog"]

    args = {k: np.asarray(inputs[k], np.float32)
            for k in ("x", "wq", "bq", "wk", "bk", "wv", "bv", "wc", "bc")}
    in_maps = _prep_inputs(**args)
    res = run_bass_kernel_spmd(nc, in_maps, core_ids=list(range(NCORES)),
                               trace=TRACE)
    LAST_RESULT = res

    # V-bias contribution: att gets +bv per head dim, so y gets +bv @ wc.T
    bc_eff = args["bc"] + args["bv"] @ args["wc"].T
    out = np.empty((S, B, D), np.float32)
    for b in range(B):
        acc = res.results[4 * b]["y"].astype(np.float32)
        for c in range(4 * b + 1, 4 * b + 4):
            acc = acc + res.results[c]["y"]
        out[:, b, :] = acc + bc_eff[None, :]
    return out
